# revision 1
# baseline (speedup 1.0000x reference)
"""Trainium2 Bass kernel for nn_CrossAttention_24438363914471.

Cross-attention module: B=8, C=512, H=W=48 (N=2304 tokens per batch image).
Reference computation per batch b:
    q = lf^T Wq^T + bq ; k = gf^T Wk^T + bk ; v = gf^T Wv^T + bv
    attn = softmax(q k^T) ; out = attn v ; out = out Wo^T + bo
    result = lf + out^T ; output = Wconv . result + bconv      # 1x1 conv C->1

Because the final 1x1 conv collapses all C channels into one scalar per pixel,
nearly everything folds (computed host-side, weights only — no activations):
    A      = Wq^T Wk                 (then S = lf^T A gf + rowterm + q-only terms)
    rowterm= (Wk^T bq)^T gf          (k-dependent softmax bias; q-only terms cancel)
    weff   = Wo^T Wconv^T            ->  wv = Wv^T weff  (so  Wconv.(Wo attn_v) =
                                          sum_k p_k (wv.gf_k) / sum_k p_k + consts)
    out[q] = Wconv.lf_q + num[q]/den[q] + (weff.bv + Wconv.bo + bconv)

Device work per core (1 batch element, data-parallel over B across 8 cores):
    U  = A gf                                  [512,2304]   96 matmuls
    T0 = U^T lf  (attention logits^T)          [2304,2304] 432 matmuls
    P  = exp(T0 + rowterm - CM)   (ACT engine, constant shift CM: softmax is
                                   shift-invariant; CM only prevents overflow)
    [num;den] = [vw|1]^T P                     [2,2304]    108 matmuls
plus tiny vector matmuls (rowterm, vw.gf, Wconv.lf) and an O(N) epilogue.
Logit-path matmuls run in fp16 (fp32 lowers to 2 slow LOW_HIGH passes on the
PE; fp16 is single-pass at N/2.4GHz), exp/num-den in bf16 (fp16 would
overflow at exp values up to e^37). num/den accumulate in fp32 PSUM.
FP8 was evaluated numerically and rejected: logit std is ~22 so the softmax is
extremely peaked; e4m3 rounding of lf/U adds ~0.5 abs logit noise which
reshuffles the top keys (rel err 0.4-0.8 vs the 2e-2 gate).
PE column-tiling (tile_position) of the [128,2]-stationary num/den matmuls was
tried and measured ZERO concurrency (each col-group still pays a full moving
pass when the moving operands differ) — reverted.

Perf structure (vs the 141.8us v1 baseline):
  * 32 dummy warm-up matmuls (~3.4us = one full HAM window) on a memset
    scratch tile run during the initial DMA wait so the PE clock-gate is at
    8/8 (2.4GHz) when real matmuls start (v1 ran ~9us of matmuls at 1.2GHz).
  * per chunk, 1b (rowterm/vw.gf) runs BEFORE 1a so the last rowterm store
    lands early; the [2,HW]->[128,18] transpose round-trip through DRAM then
    overlaps the tail of phase 1 instead of stalling phase 2's first exp.
  * the epilogue stays in ROW space (q on the free axis): num/den partials
    are divided and added to convlf as [1..2,w] rows per chunk, then stored
    straight to out[q0:q0+w] (contiguous DMA) per chunk.  The tail after the
    last matmul is only the last (smallest) chunk's epilogue, not a full
    [2,2304] DRAM round-trip + transpose + gather (v1 tail was ~6us).
  * convlf (1c) output never leaves row space (it lands in clf_row and is
    consumed by the row-space epilogue) — no reshape round-trip for it.
"""

import numpy as np
from contextlib import ExitStack

import concourse.bass as bass
import concourse.tile as tile
from concourse import bacc, mybir
from concourse.bass_utils import run_bass_kernel_spmd
from concourse.tile import add_dep_helper

F32 = mybir.dt.float32
F16 = mybir.dt.float16
BF16 = mybir.dt.bfloat16
P = 128                 # partitions
C = 512                 # channels
HW = 2304               # tokens per batch (48*48)
NCT = C // P            # 4 channel tiles
NKT = HW // P           # 18 key tiles
NCORES = 8
CHUNKS = [(0, 256), (256, 512), (768, 512), (1280, 512), (1792, 256), (2048, 256)]
CM = 105.0              # constant softmax shift (true row maxes are ~57..142)
NWARM = 64              # warm-up matmuls (N=128 each): the first ~32 span one
                        # HAM window at 1.2GHz (~3.4us), the rest run at
                        # 2.4GHz; total ~4.8us.  Sized so real matmuls start
                        # only once the gf DMA stream has built enough lead
                        # for warm-rate phase 1 to never starve (starvation
                        # gaps re-throttle the clock-gate, costing ~2x more).

_EXP = mybir.ActivationFunctionType.Exp
_ADD = mybir.AluOpType.add


def _build_program(const_add: float) -> bacc.Bacc:
    nc = bacc.Bacc("TRN2", target_bir_lowering=False, debug=False)

    lf_d = nc.dram_tensor("lf", (NCT, P, HW), F16, kind="ExternalInput").ap()
    gf_d = nc.dram_tensor("gf", (NCT, P, HW), F16, kind="ExternalInput").ap()
    at_d = nc.dram_tensor("at", (P, NCT, NCT, P), F16, kind="ExternalInput").ap()
    vecs_d = nc.dram_tensor("vecs", (P, NCT, 3), F16, kind="ExternalInput").ap()
    vtmp = nc.dram_tensor("vtmp", (2, HW), F32, kind="Internal").ap()
    out_d = nc.dram_tensor("out", (HW,), F32, kind="ExternalOutput").ap()

    with tile.TileContext(nc) as tc, ExitStack() as ctx:
        big = ctx.enter_context(tc.tile_pool(name="big", bufs=1))
        small = ctx.enter_context(tc.tile_pool(name="small", bufs=1))
        ppool = ctx.enter_context(tc.tile_pool(name="pp", bufs=20))
        stg = ctx.enter_context(tc.tile_pool(name="stg", bufs=2))
        rows = ctx.enter_context(tc.tile_pool(name="rows", bufs=3))
        psA = ctx.enter_context(tc.tile_pool(name="psA", bufs=6, space="PSUM"))
        psB = ctx.enter_context(tc.tile_pool(name="psB", bufs=2, space="PSUM"))

        gf_sb = big.tile([P, NCT, HW], F16, tag="gf")
        lf_sb = big.tile([P, NCT, HW], F16, tag="lf")
        u_sb = big.tile([P, NCT, HW], F16, tag="u")
        at_sb = small.tile([P, NCT, NCT, P], F16, tag="at")
        vecs_sb = small.tile([P, NCT, 3], F16, tag="vecs")
        wtile = small.tile([P, P], F16, tag="warm")
        clf_row = small.tile([1, HW], F32, tag="clf")    # convlf, row space

        r_sb = small.tile([P, NKT], F32, tag="r")
        vwg32 = small.tile([P, NKT], F32, tag="vwg")
        biasR = small.tile([P, NKT], F32, tag="biasR")
        vwones = small.tile([P, 2, NKT], BF16, tag="vwones")


        # ---- warm-up: memset a scratch tile, then NWARM dummy matmuls so the
        # PE HAM clock-gate reaches 8/8 (2.4GHz) during the initial DMA wait.
        nc.gpsimd.memset(wtile, 0.015625)
        wps = psB.tile([P, P], F32, tag="nd")
        for _ in range(NWARM):
            nc.tensor.matmul(wps, wtile, wtile, start=True, stop=True)

        nc.vector.memset(vwones[:, 1:2, :], 1.0)

        # ---- input DMAs.  Priority order: the 4 gf[0:256] pieces FIRST
        # (chunk 0's 1b+1a gate on them; queueing them behind at/vecs cost a
        # HAM re-throttle), then vecs + at on gpsimd, then the rest of gf
        # round-robin on all 3 queues, then all of lf (first needed by phase
        # 1c, ~2/3 into phase 1).
        nc.scalar.dma_start(gf_sb[:, 0, 0:256], gf_d[0][:, 0:256])
        nc.sync.dma_start(gf_sb[:, 1, 0:256], gf_d[1][:, 0:256])
        nc.scalar.dma_start(gf_sb[:, 2, 0:256], gf_d[2][:, 0:256])
        nc.sync.dma_start(gf_sb[:, 3, 0:256], gf_d[3][:, 0:256])
        nc.scalar.dma_start(at_sb[:, 0:1], at_d[:, 0:1])
        nc.sync.dma_start(at_sb[:, 1:4], at_d[:, 1:4])
        nc.gpsimd.dma_start(vecs_sb, vecs_d)
        # gf+at ride ONLY the two fast HWDGE queues (phase 1 consumes gf at
        # ~180GB/s when warm; the gpsimd SWDGE runs at ~80GB/s and putting
        # anything phase-1-critical on it re-throttles the PE clock-gate).
        # All of lf rides gpsimd: it is first needed by phase 1c (~2/3 into
        # phase 1) and trickles in comfortably by then.
        ENGS = (nc.scalar, nc.sync, nc.gpsimd)
        ei = 0
        GSLICES = [(256, 512), (768, 512), (1280, 512), (1792, 512)]
        for h0, hw_ in GSLICES:
            for t in range(NCT):
                eng = ENGS[ei % 3]
                ei += 1
                eng.dma_start(gf_sb[:, t, h0 : h0 + hw_], gf_d[t][:, h0 : h0 + hw_])
        LSLICES = [(0, 768), (768, 768), (1536, 768)]
        for h0, hw_ in LSLICES:
            for t in range(NCT):
                eng = ENGS[ei % 3]
                ei += 1
                eng.dma_start(lf_sb[:, t, h0 : h0 + hw_], lf_d[t][:, h0 : h0 + hw_])

        # ---- phase 1 per chunk: 1b (rowterm/vw.gf) FIRST so the reshape
        # round-trip overlaps the rest of phase 1, then 1a (U = A gf).
        # ALL phase-1 PSUM->SBUF copies run on the VECTOR engine: the scalar
        # engine spends phase 1 issuing the input DMAs (each dma_start is
        # ~590ns + ring-backpressure waits on its HWDGE ring); putting the U
        # copies on it starved psA and froze the PE for ~5us.
        vec_stores = []
        for ci_, (q0, w) in enumerate(CHUNKS):
            ps2 = psB.tile([2, w], F32, tag="nd")
            for ci in range(NCT):
                nc.tensor.matmul(
                    ps2,
                    vecs_sb[:, ci, 0:2],
                    gf_sb[:, ci, q0 : q0 + w],
                    start=(ci == 0),
                    stop=(ci == NCT - 1),
                )
            st = stg.tile([2, w], F32, tag="vstage")
            nc.vector.tensor_copy(st, ps2)
            eng = nc.sync if ci_ % 2 == 0 else nc.gpsimd
            vec_stores.append(eng.dma_start(vtmp[:, q0 : q0 + w], st))

            for co in range(NCT):
                ps = psA.tile([P, w], F32, tag="ps")
                for ci in range(NCT):
                    nc.tensor.matmul(
                        ps,
                        at_sb[:, co, ci, :],
                        gf_sb[:, ci, q0 : q0 + w],
                        start=(ci == 0),
                        stop=(ci == NCT - 1),
                    )
                nc.vector.tensor_copy(u_sb[:, co, q0 : q0 + w], ps)

        # ---- reshape rowterm / vw.gf into [128,18] partition-major tiles
        # (q = t*128 + p bijection) and build the per-key exp bias.
        ld = nc.sync.dma_start(r_sb, vtmp[0].rearrange("(t p) -> p t", p=P))
        for s in vec_stores:
            add_dep_helper(ld.ins, s.ins, reason="dram raw rowterm")
        ld = nc.gpsimd.dma_start(vwg32, vtmp[1].rearrange("(t p) -> p t", p=P))
        for s in vec_stores:
            add_dep_helper(ld.ins, s.ins, reason="dram raw vwgf")
        nc.vector.tensor_scalar_add(biasR, r_sb, -CM)
        nc.vector.tensor_copy(vwones[:, 0:1, :], vwg32)

        # ---- phase 1c: convlf = Wconv . lf -> clf_row (stays in row space)
        for ci_, (q0, w) in enumerate(CHUNKS):
            ps3 = psB.tile([2, w], F32, tag="nd")
            for ci in range(NCT):
                nc.tensor.matmul(
                    ps3[0:1, :],
                    vecs_sb[:, ci, 2:3],
                    lf_sb[:, ci, q0 : q0 + w],
                    start=(ci == 0),
                    stop=(ci == NCT - 1),
                )
            nc.vector.tensor_copy(clf_row[0:1, q0 : q0 + w], ps3[0:1, :])

        # prefetch the per-chunk partition-major convlf tiles now (phase 2
        # must not issue DMAs on the scalar engine: they interleave with and
        # delay the exp stream).
        clf_cs = []
        for ci_, (q0, w) in enumerate(CHUNKS):
            nt = w // P
            clf_c = small.tile([P, nt], F32, tag=f"clfc{ci_}")
            nc.scalar.dma_start(
                clf_c, clf_row[0:1, q0 : q0 + w].rearrange("r (p t) -> r p t", t=nt)
            )
            clf_cs.append(clf_c)


        # ---- phase 2 per chunk: logits + exp for all 18 k-tiles, then the 18
        # num/den matmuls back-to-back (batching bf16 after fp16 avoids the
        # ~95ns PE dtype-switch penalty at every tile boundary).  Division +
        # convlf add happen in row space; result DMAs straight to out[q0:].
        for ci_, (q0, w) in enumerate(CHUNKS):
            pexps = []
            for kt in range(NKT):
                t0 = psA.tile([P, w], F32, tag="ps")
                for ct in range(NCT):
                    nc.tensor.matmul(
                        t0,
                        u_sb[:, ct, kt * P : (kt + 1) * P],
                        lf_sb[:, ct, q0 : q0 + w],
                        start=(ct == 0),
                        stop=(ct == NCT - 1),
                    )
                pexp = ppool.tile([P, w], BF16, tag="pexp")
                nc.scalar.activation(
                    pexp, t0, _EXP, bias=biasR[:, kt : kt + 1], scale=1.0
                )
                pexps.append(pexp)

            nd = psB.tile([2, w], F32, tag="nd")
            for kt in range(NKT):
                nc.tensor.matmul(
                    nd,
                    vwones[:, :, kt : kt + 1],
                    pexps[kt],
                    start=(kt == 0),
                    stop=(kt == NKT - 1),
                )

            # incremental epilogue with a PER-CHUNK p-major bijection
            # q = q0 + p*nt + t: every DMA gather/scatter then moves nt
            # contiguous f32 per partition (128 descriptors), not a 4-byte
            # scatter (the global t*128+p bijection was measured at ~10ns per
            # element of queue occupancy and jammed the DMA queues).
            # Division happens on 128 partitions (row-space reciprocal is ~7
            # cyc/elem on a single lane = 1.7us/chunk — measured).
            nt = w // P
            nd2 = rows.tile([2, w], F32, tag="nd2")
            nc.vector.tensor_copy(nd2, nd)
            ndn = rows.tile([P, nt], F32, tag="ndn")
            ndd = rows.tile([P, nt], F32, tag="ndd")
            clf_c = clf_cs[ci_]
            # SBUF->SBUF gathers: legal because the p-major view keeps the
            # final AP dim contiguous within 3 dims (the t*128+p view did
            # not), and ~100x fewer descriptors than a 4-byte scatter.
            nc.sync.dma_start(ndn, nd2[0:1, :].rearrange("r (p t) -> r p t", t=nt))
            nc.sync.dma_start(ndd, nd2[1:2, :].rearrange("r (p t) -> r p t", t=nt))
            rec = rows.tile([P, nt], F32, tag="rec")
            nc.vector.reciprocal(rec, ndd)
            nc.vector.tensor_mul(rec, ndn, rec)
            fin_c = rows.tile([P, nt], F32, tag="fin")
            nc.vector.scalar_tensor_tensor(
                fin_c, rec, float(const_add), clf_c, op0=_ADD, op1=_ADD,
            )
            nc.sync.dma_start(
                out_d[q0 : q0 + w].rearrange("(p t) -> p t", t=nt), fin_c
            )

    nc.compile()
    return nc


_CACHE: dict[bytes, bacc.Bacc] = {}


def _fold(inputs):
    f64 = np.float64
    Wq, bq = inputs["Wq"].astype(f64), inputs["bq"].astype(f64)
    Wk, bk = inputs["Wk"].astype(f64), inputs["bk"].astype(f64)
    Wv, bv = inputs["Wv"].astype(f64), inputs["bv"].astype(f64)
    Wo, bo = inputs["Wo"].astype(f64), inputs["bo"].astype(f64)
    Wconv, bconv = inputs["Wconv"].astype(f64), inputs["bconv"].astype(f64)

    A = Wq.T @ Wk                       # S0 = lf^T A gf
    AT = np.ascontiguousarray(
        A.T.astype(np.float16).reshape(NCT, P, NCT, P).transpose(1, 2, 0, 3)
    )
    wkb = Wk.T @ bq                     # rowterm = wkb^T gf
    weff = Wo.T @ Wconv[0]
    wv = Wv.T @ weff
    vecs = np.stack(
        [wkb.astype(np.float32), wv.astype(np.float32), inputs["Wconv"][0]], axis=1
    )                                   # [C, 3]
    vecs = np.ascontiguousarray(
        vecs.astype(np.float16).reshape(NCT, P, 3).transpose(1, 0, 2)
    )
    const_add = float(weff @ bv + Wconv[0] @ bo + bconv[0])
    return AT, vecs, const_add


def _prepare_in_maps(inputs):
    AT, vecs, const_add = _fold(inputs)
    lf = np.ascontiguousarray(inputs["local_feat"].astype(np.float16)).reshape(
        NCORES, NCT, P, HW
    )
    gf = np.ascontiguousarray(inputs["global_feat"].astype(np.float16)).reshape(
        NCORES, NCT, P, HW
    )
    in_maps = [
        {"lf": lf[b], "gf": gf[b], "at": AT, "vecs": vecs} for b in range(NCORES)
    ]
    return in_maps, const_add


def run(inputs, trace: bool = False, **kwargs):
    """Run on hardware; returns (output [8,1,48,48], BassKernelResults)."""
    in_maps, const_add = _prepare_in_maps(inputs)
    key = np.float32(const_add).tobytes()
    if key not in _CACHE:
        _CACHE[key] = _build_program(const_add)
    nc = _CACHE[key]
    res = run_bass_kernel_spmd(
        nc, in_maps, core_ids=list(range(NCORES)), trace=trace, **kwargs
    )
    out = np.stack([res.results[b]["out"] for b in range(NCORES)], axis=0)
    return out.reshape(NCORES, 1, 48, 48).astype(np.float32), res


def kernel(**inputs) -> np.ndarray:
    out, _ = run(inputs)
    return out



# revision 10
# speedup vs baseline: 1.0243x; 1.0243x over previous
"""Trainium2 Bass kernel for nn_CrossAttention_24438363914471.

Cross-attention module: B=8, C=512, H=W=48 (N=2304 tokens per batch image).
Reference computation per batch b:
    q = lf^T Wq^T + bq ; k = gf^T Wk^T + bk ; v = gf^T Wv^T + bv
    attn = softmax(q k^T) ; out = attn v ; out = out Wo^T + bo
    result = lf + out^T ; output = Wconv . result + bconv      # 1x1 conv C->1

Because the final 1x1 conv collapses all C channels into one scalar per pixel,
nearly everything folds (computed host-side, weights only — no activations):
    A      = Wq^T Wk                 (then S = lf^T A gf + rowterm + q-only terms)
    rowterm= (Wk^T bq)^T gf          (k-dependent softmax bias; q-only terms cancel)
    weff   = Wo^T Wconv^T            ->  wv = Wv^T weff  (so  Wconv.(Wo attn_v) =
                                          sum_k p_k (wv.gf_k) / sum_k p_k + consts)
    out[q] = Wconv.lf_q + num[q]/den[q] + (weff.bv + Wconv.bo + bconv)

Device work per core (1 batch element, data-parallel over B across 8 cores):
    U  = A gf                                  [512,2304]   96 matmuls
    T0 = U^T lf  (attention logits^T)          [2304,2304] 432 matmuls
    P  = exp(T0 + rowterm - CM)   (ACT engine, constant shift CM: softmax is
                                   shift-invariant; CM only prevents overflow)
    [num;den] = [vw|1]^T P                     [2,2304]    108 matmuls
plus tiny vector matmuls (rowterm, vw.gf, Wconv.lf) and an O(N) epilogue.
Logit-path matmuls run in fp16 (fp32 lowers to 2 slow LOW_HIGH passes on the
PE; fp16 is single-pass at N/2.4GHz), exp/num-den in bf16 (fp16 would
overflow at exp values up to e^37). num/den accumulate in fp32 PSUM.
FP8 was evaluated numerically and rejected: logit std is ~22 so the softmax is
extremely peaked; e4m3 rounding of lf/U adds ~0.5 abs logit noise which
reshuffles the top keys (rel err 0.4-0.8 vs the 2e-2 gate).
PE column-tiling (tile_position) of the [128,2]-stationary num/den matmuls was
tried and measured ZERO concurrency (each col-group still pays a full moving
pass when the moving operands differ) — reverted.

Perf structure (vs the 141.8us v1 baseline):
  * 32 dummy warm-up matmuls (~3.4us = one full HAM window) on a memset
    scratch tile run during the initial DMA wait so the PE clock-gate is at
    8/8 (2.4GHz) when real matmuls start (v1 ran ~9us of matmuls at 1.2GHz).
  * per chunk, 1b (rowterm/vw.gf) runs BEFORE 1a so the last rowterm store
    lands early; the [2,HW]->[128,18] transpose round-trip through DRAM then
    overlaps the tail of phase 1 instead of stalling phase 2's first exp.
  * the epilogue stays in ROW space (q on the free axis): num/den partials
    are divided and added to convlf as [1..2,w] rows per chunk, then stored
    straight to out[q0:q0+w] (contiguous DMA) per chunk.  The tail after the
    last matmul is only the last (smallest) chunk's epilogue, not a full
    [2,2304] DRAM round-trip + transpose + gather (v1 tail was ~6us).
  * convlf (1c) output never leaves row space (it lands in clf_row and is
    consumed by the row-space epilogue) — no reshape round-trip for it.
"""

import numpy as np
from contextlib import ExitStack

import concourse.bass as bass
import concourse.tile as tile
from concourse import bacc, mybir
from concourse.bass_utils import run_bass_kernel_spmd
from concourse.tile import add_dep_helper

F32 = mybir.dt.float32
F16 = mybir.dt.float16
BF16 = mybir.dt.bfloat16
P = 128                 # partitions
C = 512                 # channels
HW = 2304               # tokens per batch (48*48)
NCT = C // P            # 4 channel tiles
NKT = HW // P           # 18 key tiles
NCORES = 8
CHUNKS = [(0, 256), (256, 512), (768, 512), (1280, 512), (1792, 256), (2048, 256)]
CM = 105.0              # constant softmax shift (true row maxes are ~57..142)
NWARM = 40              # warm-up matmuls (N=128 each): the first ~32 span one
                        # HAM window at 1.2GHz (~3.4us), the rest run at
                        # 2.4GHz.  Sized so real matmuls start only once the
                        # gf DMA stream has built enough lead for warm-rate
                        # phase 1 to never starve (starvation gaps re-throttle
                        # the clock-gate, costing ~2x more).

_EXP = mybir.ActivationFunctionType.Exp
_ADD = mybir.AluOpType.add


def _build_program(const_add: float) -> bacc.Bacc:
    nc = bacc.Bacc("TRN2", target_bir_lowering=False, debug=False)

    lf_d = nc.dram_tensor("lf", (NCT, P, HW), F16, kind="ExternalInput").ap()
    gf_d = nc.dram_tensor("gf", (NCT, P, HW), F16, kind="ExternalInput").ap()
    at_d = nc.dram_tensor("at", (P, NCT, NCT, P), F16, kind="ExternalInput").ap()
    vecs_d = nc.dram_tensor("vecs", (P, NCT, 3), F16, kind="ExternalInput").ap()
    eye_d = nc.dram_tensor("eye", (P, P), F32, kind="ExternalInput").ap()
    vtmp = nc.dram_tensor("vtmp", (2, HW), F32, kind="Internal").ap()
    out_d = nc.dram_tensor("out", (HW,), F32, kind="ExternalOutput").ap()

    with tile.TileContext(nc) as tc, ExitStack() as ctx:
        big = ctx.enter_context(tc.tile_pool(name="big", bufs=1))
        small = ctx.enter_context(tc.tile_pool(name="small", bufs=1))
        ppool = ctx.enter_context(tc.tile_pool(name="pp", bufs=20))
        stg = ctx.enter_context(tc.tile_pool(name="stg", bufs=2))
        rows = ctx.enter_context(tc.tile_pool(name="rows", bufs=3))
        psA = ctx.enter_context(tc.tile_pool(name="psA", bufs=5, space="PSUM"))
        psB = ctx.enter_context(tc.tile_pool(name="psB", bufs=2, space="PSUM"))
        psT = ctx.enter_context(tc.tile_pool(name="psT", bufs=1, space="PSUM"))

        gf_sb = big.tile([P, NCT, HW], F16, tag="gf")
        lf_sb = big.tile([P, NCT, HW], F16, tag="lf")
        u_sb = big.tile([P, NCT, HW], F16, tag="u")
        at_sb = small.tile([P, NCT, NCT, P], F16, tag="at")
        vecs_sb = small.tile([P, NCT, 3], F16, tag="vecs")
        eye_sb = small.tile([P, P], F32, tag="eye")
        wtile = small.tile([P, P], F16, tag="warm")
        clf_row = small.tile([1, HW], F32, tag="clf")    # convlf, row space

        r_sb = small.tile([P, NKT], F32, tag="r")
        vwg32 = small.tile([P, NKT], F32, tag="vwg")
        biasR = small.tile([P, NKT], F32, tag="biasR")
        vwones = small.tile([P, 2, NKT], BF16, tag="vwones")


        # ---- warm-up: memset a scratch tile, then NWARM dummy matmuls so the
        # PE HAM clock-gate reaches 8/8 (2.4GHz) during the initial DMA wait.
        # memset on VECTOR: gpsimd took ~4us to run it, stalling the warm-up
        # (and the vecs DMA queued behind it) until ts~7.4us.
        nc.vector.memset(wtile, 0.015625)
        wps = psB.tile([P, P], F32, tag="nd")
        for _ in range(NWARM):
            nc.tensor.matmul(wps, wtile, wtile, start=True, stop=True)

        nc.vector.memset(vwones[:, 1:2, :], 1.0)

        # ---- input DMAs.  Priority order: the 4 gf[0:256] pieces FIRST
        # (chunk 0's 1b+1a gate on them; queueing them behind at/vecs cost a
        # HAM re-throttle), then vecs + at on gpsimd, then the rest of gf
        # round-robin on all 3 queues, then all of lf (first needed by phase
        # 1c, ~2/3 into phase 1).
        nc.scalar.dma_start(gf_sb[:, 0, 0:256], gf_d[0][:, 0:256])
        nc.sync.dma_start(gf_sb[:, 1, 0:256], gf_d[1][:, 0:256])
        nc.scalar.dma_start(gf_sb[:, 2, 0:256], gf_d[2][:, 0:256])
        nc.sync.dma_start(gf_sb[:, 3, 0:256], gf_d[3][:, 0:256])
        nc.scalar.dma_start(at_sb[:, 0:1], at_d[:, 0:1])
        nc.sync.dma_start(at_sb[:, 1:4], at_d[:, 1:4])
        nc.gpsimd.dma_start(vecs_sb, vecs_d)
        nc.gpsimd.dma_start(eye_sb, eye_d)   # only needed by the tail epilogue
        # gf+at ride ONLY the two fast HWDGE queues (phase 1 consumes gf at
        # ~180GB/s when warm; the gpsimd SWDGE runs at ~80GB/s and putting
        # anything phase-1-critical on it re-throttles the PE clock-gate).
        # All of lf rides gpsimd: it is first needed by phase 1c (~2/3 into
        # phase 1) and trickles in comfortably by then.
        ENGS = (nc.scalar, nc.sync, nc.gpsimd)
        ei = 0
        GSLICES = [(256, 512), (768, 512), (1280, 512), (1792, 512)]
        for h0, hw_ in GSLICES:
            for t in range(NCT):
                eng = ENGS[ei % 3]
                ei += 1
                eng.dma_start(gf_sb[:, t, h0 : h0 + hw_], gf_d[t][:, h0 : h0 + hw_])
        LSLICES = [(0, 768), (768, 768), (1536, 768)]
        for h0, hw_ in LSLICES:
            for t in range(NCT):
                eng = ENGS[ei % 3]
                ei += 1
                eng.dma_start(lf_sb[:, t, h0 : h0 + hw_], lf_d[t][:, h0 : h0 + hw_])

        # ---- phase 1 per chunk: 1b (rowterm/vw.gf) FIRST so the reshape
        # round-trip overlaps the rest of phase 1, then 1a (U = A gf).
        # ALL phase-1 PSUM->SBUF copies run on the VECTOR engine: the scalar
        # engine spends phase 1 issuing the input DMAs (each dma_start is
        # ~590ns + ring-backpressure waits on its HWDGE ring); putting the U
        # copies on it starved psA and froze the PE for ~5us.
        vec_stores = []
        for ci_, (q0, w) in enumerate(CHUNKS):
            ps2 = psB.tile([2, w], F32, tag="nd")
            for ci in range(NCT):
                nc.tensor.matmul(
                    ps2,
                    vecs_sb[:, ci, 0:2],
                    gf_sb[:, ci, q0 : q0 + w],
                    start=(ci == 0),
                    stop=(ci == NCT - 1),
                )
            st = stg.tile([2, w], F32, tag="vstage")
            nc.vector.tensor_copy(st, ps2)
            eng = nc.sync if ci_ % 2 == 0 else nc.gpsimd
            vec_stores.append(eng.dma_start(vtmp[:, q0 : q0 + w], st))

            for co in range(NCT):
                ps = psA.tile([P, w], F32, tag="ps")
                for ci in range(NCT):
                    nc.tensor.matmul(
                        ps,
                        at_sb[:, co, ci, :],
                        gf_sb[:, ci, q0 : q0 + w],
                        start=(ci == 0),
                        stop=(ci == NCT - 1),
                    )
                nc.vector.tensor_copy(u_sb[:, co, q0 : q0 + w], ps)

        # ---- reshape rowterm / vw.gf into [128,18] partition-major tiles
        # (q = t*128 + p bijection) and build the per-key exp bias.
        ld = nc.sync.dma_start(r_sb, vtmp[0].rearrange("(t p) -> p t", p=P))
        for s in vec_stores:
            add_dep_helper(ld.ins, s.ins, reason="dram raw rowterm")
        ld = nc.gpsimd.dma_start(vwg32, vtmp[1].rearrange("(t p) -> p t", p=P))
        for s in vec_stores:
            add_dep_helper(ld.ins, s.ins, reason="dram raw vwgf")
        nc.vector.tensor_scalar_add(biasR, r_sb, -CM)
        nc.vector.tensor_copy(vwones[:, 0:1, :], vwg32)

        # ---- phase 1c: convlf = Wconv . lf -> clf_row (stays in row space)
        for ci_, (q0, w) in enumerate(CHUNKS):
            ps3 = psB.tile([2, w], F32, tag="nd")
            for ci in range(NCT):
                nc.tensor.matmul(
                    ps3[0:1, :],
                    vecs_sb[:, ci, 2:3],
                    lf_sb[:, ci, q0 : q0 + w],
                    start=(ci == 0),
                    stop=(ci == NCT - 1),
                )
            nc.vector.tensor_copy(clf_row[0:1, q0 : q0 + w], ps3[0:1, :])

        # prefetch the per-chunk partition-major convlf tiles now (phase 2
        # must not issue DMAs on the scalar engine: they interleave with and
        # delay the exp stream).
        clf_cs = []
        for ci_, (q0, w) in enumerate(CHUNKS[:-1]):
            nt = w // P
            clf_c = small.tile([P, nt], F32, tag=f"clfc{ci_}")
            nc.scalar.dma_start(
                clf_c, clf_row[0:1, q0 : q0 + w].rearrange("r (p t) -> r p t", t=nt)
            )
            clf_cs.append(clf_c)
        clf_cs.append(None)  # last chunk uses the PE-transpose tail instead


        # ---- phase 2 per chunk: logits + exp for all 18 k-tiles, then the 18
        # num/den matmuls back-to-back (batching bf16 after fp16 avoids the
        # ~95ns PE dtype-switch penalty at every tile boundary).  Division +
        # convlf add happen in row space; result DMAs straight to out[q0:].
        for ci_, (q0, w) in enumerate(CHUNKS):
            pexps = []
            for kt in range(NKT):
                t0 = psA.tile([P, w], F32, tag="ps")
                for ct in range(NCT):
                    nc.tensor.matmul(
                        t0,
                        u_sb[:, ct, kt * P : (kt + 1) * P],
                        lf_sb[:, ct, q0 : q0 + w],
                        start=(ct == 0),
                        stop=(ct == NCT - 1),
                    )
                pexp = ppool.tile([P, w], BF16, tag="pexp")
                nc.scalar.activation(
                    pexp, t0, _EXP, bias=biasR[:, kt : kt + 1], scale=1.0
                )
                pexps.append(pexp)

            nd = psB.tile([2, w], F32, tag="nd")
            for kt in range(NKT):
                nc.tensor.matmul(
                    nd,
                    vwones[:, :, kt : kt + 1],
                    pexps[kt],
                    start=(kt == 0),
                    stop=(kt == NKT - 1),
                )

            nd2 = rows.tile([2, w], F32, tag="nd2")
            nc.vector.tensor_copy(nd2, nd)
            if ci_ < len(CHUNKS) - 1:
                # incremental epilogue with a PER-CHUNK p-major bijection
                # q = q0 + p*nt + t: every DMA gather/scatter then moves nt
                # contiguous f32 per partition (128 descriptors), not a 4-byte
                # scatter (the global t*128+p bijection was measured at ~10ns
                # per element of queue occupancy and jammed the DMA queues).
                # Division happens on 128 partitions (row-space reciprocal is
                # ~7 cyc/elem on a single lane = 1.7us/chunk — measured).
                nt = w // P
                ndn = rows.tile([P, nt], F32, tag="ndn")
                ndd = rows.tile([P, nt], F32, tag="ndd")
                clf_c = clf_cs[ci_]
                # SBUF->SBUF gathers: legal because the p-major view keeps the
                # final AP dim contiguous within 3 dims (the t*128+p view did
                # not), and ~100x fewer descriptors than a 4-byte scatter.
                nc.sync.dma_start(ndn, nd2[0:1, :].rearrange("r (p t) -> r p t", t=nt))
                nc.sync.dma_start(ndd, nd2[1:2, :].rearrange("r (p t) -> r p t", t=nt))
                rec = rows.tile([P, nt], F32, tag="rec")
                nc.vector.reciprocal(rec, ndd)
                nc.vector.tensor_mul(rec, ndn, rec)
                fin_c = rows.tile([P, nt], F32, tag="fin")
                nc.vector.scalar_tensor_tensor(
                    fin_c, rec, float(const_add), clf_c, op0=_ADD, op1=_ADD,
                )
                nc.sync.dma_start(
                    out_d[q0 : q0 + w].rearrange("(p t) -> p t", t=nt), fin_c
                )
            else:
                # LAST chunk: the old gather path put ~6us of DMA latency on
                # the critical tail (SBUF->SBUF gather round-trip + a
                # 128-descriptor scatter store).  Instead transpose on the PE
                # (in_^T @ I): nd [2,128]-blocks -> [128,2], divide on 128
                # partitions, transpose back -> [nb,128] and store with nb
                # contiguous 512B descriptors.
                nb = w // P
                tp = psT.tile([P, 160], F32, tag="tp")
                ndT = tp[:, 0:4].rearrange("p (j t) -> p j t", t=2)
                for j in range(nb):
                    nc.tensor.transpose(
                        ndT[:, j : j + 1, :],
                        nd2[:, j * P : (j + 1) * P],
                        eye_sb[0:2, 0:2],
                    )
                clfT = tp[:, 4 : 4 + nb]
                for j in range(nb):
                    nc.tensor.transpose(
                        clfT[:, j : j + 1],
                        clf_row[0:1, q0 + j * P : q0 + (j + 1) * P],
                        eye_sb[0:1, 0:1],
                    )
                rec = rows.tile([P, nb], F32, tag="rec")
                nc.vector.reciprocal(rec, ndT[:, :, 1])
                fin_c = rows.tile([P, nb], F32, tag="fin")
                nc.vector.tensor_mul(fin_c, ndT[:, :, 0], rec)
                nc.vector.scalar_tensor_tensor(
                    fin_c, fin_c, float(const_add), clfT, op0=_ADD, op1=_ADD,
                )
                finT = tp[0:nb, 32 : 32 + P]     # [nb, 128] in PSUM
                nc.tensor.transpose(finT, fin_c, eye_sb)
                fin_sb = rows.tile([nb, P], F32, tag="finrow")
                nc.vector.tensor_copy(fin_sb, finT)
                nc.sync.dma_start(
                    out_d[q0 : q0 + w].rearrange("(t p) -> t p", p=P), fin_sb
                )

    nc.compile()
    return nc


_CACHE: dict[bytes, bacc.Bacc] = {}


def _fold(inputs):
    f64 = np.float64
    Wq, bq = inputs["Wq"].astype(f64), inputs["bq"].astype(f64)
    Wk, bk = inputs["Wk"].astype(f64), inputs["bk"].astype(f64)
    Wv, bv = inputs["Wv"].astype(f64), inputs["bv"].astype(f64)
    Wo, bo = inputs["Wo"].astype(f64), inputs["bo"].astype(f64)
    Wconv, bconv = inputs["Wconv"].astype(f64), inputs["bconv"].astype(f64)

    A = Wq.T @ Wk                       # S0 = lf^T A gf
    AT = np.ascontiguousarray(
        A.T.astype(np.float16).reshape(NCT, P, NCT, P).transpose(1, 2, 0, 3)
    )
    wkb = Wk.T @ bq                     # rowterm = wkb^T gf
    weff = Wo.T @ Wconv[0]
    wv = Wv.T @ weff
    vecs = np.stack(
        [wkb.astype(np.float32), wv.astype(np.float32), inputs["Wconv"][0]], axis=1
    )                                   # [C, 3]
    vecs = np.ascontiguousarray(
        vecs.astype(np.float16).reshape(NCT, P, 3).transpose(1, 0, 2)
    )
    const_add = float(weff @ bv + Wconv[0] @ bo + bconv[0])
    return AT, vecs, const_add


def _prepare_in_maps(inputs):
    AT, vecs, const_add = _fold(inputs)
    lf = np.ascontiguousarray(inputs["local_feat"].astype(np.float16)).reshape(
        NCORES, NCT, P, HW
    )
    gf = np.ascontiguousarray(inputs["global_feat"].astype(np.float16)).reshape(
        NCORES, NCT, P, HW
    )
    eye = np.eye(P, dtype=np.float32)
    in_maps = [
        {"lf": lf[b], "gf": gf[b], "at": AT, "vecs": vecs, "eye": eye}
        for b in range(NCORES)
    ]
    return in_maps, const_add


def run(inputs, trace: bool = False, **kwargs):
    """Run on hardware; returns (output [8,1,48,48], BassKernelResults)."""
    in_maps, const_add = _prepare_in_maps(inputs)
    key = np.float32(const_add).tobytes()
    if key not in _CACHE:
        _CACHE[key] = _build_program(const_add)
    nc = _CACHE[key]
    res = run_bass_kernel_spmd(
        nc, in_maps, core_ids=list(range(NCORES)), trace=trace, **kwargs
    )
    out = np.stack([res.results[b]["out"] for b in range(NCORES)], axis=0)
    return out.reshape(NCORES, 1, 48, 48).astype(np.float32), res


def kernel(**inputs) -> np.ndarray:
    out, _ = run(inputs)
    return out

